# revision 18
# baseline (speedup 1.0000x reference)
"""Trainium2 Bass kernel for batched dot-product attention + softmax.

Reference computation (all fp32):
    hidden:          [1, B=64, D=1024]
    encoder_outputs: [S=2048, B=64, D=1024]
    energies[b, s] = dot(hidden[0, b], encoder_outputs[s, b])   # [B, S]
    attn = softmax(energies, axis=-1)                           # [B, S]
    return attn[:, None, :]                                     # [B, 1, S]

Sharding: data-parallel over the batch dim -- each of the 8 NeuronCores
handles B_LOC = 8 batches. No cross-core communication (softmax is per-row).

Numerics: fp32 matmuls on the PE run at 4 cycles/row, which makes TensorE
(not HBM) the bottleneck. Instead each fp32 operand is split on the host
into a bf16 (hi, lo) pair: x = hi + lo + O(2^-18 x). Per d-chunk the PE
runs two bf16 matmuls (moving operand enc_hi then enc_lo) against the
2-column stationary [h_hi | h_lo], so PSUM rows {0,1} accumulate all four
cross products: row0 = e_hi.h_hi + e_lo.h_hi, row1 = e_hi.h_lo + e_lo.h_lo.
energies = row0 + row1 (added during the softmax stage). bf16 streams at
1 cycle/row => PE time halves; HBM traffic is unchanged (2+2 bytes/elem).

Per-core device layout (host-prepared):
    enc: [B_LOC, KG, 128, G*2, S] bf16 -- d on partitions; per (b, kg) a
         contiguous 4 MiB block holding G=4 d-chunks x (hi,lo) x S.
    h:   [128, B_LOC*DC, 2] bf16 -- (hi, lo) stationary column pairs.
"""

from contextlib import ExitStack

import numpy as np

import concourse.bacc as bacc
import concourse.bass as bass
import concourse.mybir as mybir
import concourse.tile as tile
from concourse.bass_utils import run_bass_kernel_spmd

N_CORES = 8
S = 2048
B = 64
D = 1024
P = 128
B_LOC = B // N_CORES  # 8 batches per core
DC = D // P  # 8 contraction chunks of 128
G = 4  # d-chunks per DMA (4 MiB transfers)
NBLK = 512  # moving-operand free dim per matmul (one fp32 PSUM bank)


def build_nc(
    b_loc: int = B_LOC,
    dc: int = DC,
    s: int = S,
    n_cores: int = N_CORES,
    g: int = G,
    enc_bufs: int = 3,
):
    """Build and compile the per-core Bass program (SPMD: same NEFF on all cores)."""
    assert dc % g == 0
    kg_cnt = dc // g
    nblk = min(NBLK, s)
    n_sblk = s // nblk

    nc = bacc.Bacc(
        "TRN2",
        target_bir_lowering=False,
        debug=False,
        num_devices=n_cores,
    )
    f32 = mybir.dt.float32
    bf16 = mybir.dt.bfloat16
    enc_d = nc.dram_tensor(
        "enc", [b_loc, kg_cnt, g, P, 2, s], bf16, kind="ExternalInput"
    ).ap()
    h_d = nc.dram_tensor("h", [P, b_loc * dc, 2], bf16, kind="ExternalInput").ap()
    out_d = nc.dram_tensor("out", [b_loc, s], f32, kind="ExternalOutput").ap()

    with ExitStack() as ctx:
        tc = ctx.enter_context(tile.TileContext(nc))
        enc_pool = ctx.enter_context(tc.tile_pool(name="enc_pool", bufs=enc_bufs))
        singles = ctx.enter_context(tc.tile_pool(name="singles", bufs=1))
        psum_pool = ctx.enter_context(
            tc.tile_pool(name="psum_pool", bufs=2, space="PSUM")
        )
        row_pool = ctx.enter_context(tc.tile_pool(name="row_pool", bufs=2))
        soft_pool = ctx.enter_context(tc.tile_pool(name="soft_pool", bufs=1))

        h_sb = singles.tile([P, b_loc * dc, 2], bf16)
        nc.sync.dma_start(out=h_sb, in_=h_d)

        # Alternate the two HWDGE rings (SP / ACT) across 1 MiB enc pieces.
        dma_engines = [nc.sync, nc.scalar]
        dma_idx = 0

        for b in range(b_loc):
            # psum rows {0, 1}: the M=2 stationary [h_hi | h_lo] makes each
            # moving stream (e_hi, then e_lo) hit both stationaries, so the
            # row sum holds all four cross products.
            psums = [
                psum_pool.tile([2, nblk], f32, name=f"ps_{b}_{j}", tag=f"ps{j}")
                for j in range(n_sblk)
            ]
            for kg in range(kg_cnt):
                # [128, g, 2, s] bf16 tile, filled by g 1-MiB DMA pieces so the
                # first matmuls start after ~1 MiB (subtile deps), spread over
                # both HWDGE rings for overlap.
                et = enc_pool.tile([P, g, 2, s], bf16, name=f"enc_{b}_{kg}", tag="enc")
                for gi in range(g):
                    eng = dma_engines[dma_idx % 2]
                    dma_idx += 1
                    eng.dma_start(out=et[:, gi], in_=enc_d[b, kg, gi])
                for gi in range(g):
                    k = kg * g + gi
                    col = b * dc + k
                    for j in range(n_sblk):
                        for hl in range(2):
                            nc.tensor.matmul(
                                psums[j][:, :],
                                lhsT=h_sb[:, col, :],
                                rhs=et[:, gi, hl, j * nblk : (j + 1) * nblk],
                                start=(k == 0 and hl == 0),
                                stop=(k == dc - 1 and hl == 1),
                            )
            row = row_pool.tile([2, s], f32, name=f"row_{b}", tag="row")
            for j in range(n_sblk):
                js = slice(j * nblk, (j + 1) * nblk)
                nc.vector.tensor_copy(row[:, js], psums[j])
            # fold lo row (partition 1) onto partition 0 via SBUF->SBUF DMA,
            # then run this batch's softmax entirely on partition 0 -- each
            # batch's chain overlaps the next batches' DMA/matmul stream.
            # The row max is taken from the hi row alone (lo row shifts it by
            # at most ~2^-9 |e|, which the normalization absorbs), so it runs
            # concurrently with the lo-row DMA + add. Earlier batches' 2-input
            # ops go to the otherwise-idle GpSimd so the vector engine stays
            # free for the last batch's latency-critical chain.
            last = b == b_loc - 1
            tt_eng = nc.vector if last else nc.gpsimd
            rowlo = row_pool.tile([1, s], f32, name=f"rowlo_{b}", tag="rowlo")
            nc.gpsimd.dma_start(out=rowlo, in_=row[1:2, :])
            neg_mx = row_pool.tile([1, 1], f32, name=f"mx_{b}", tag="mx")
            nc.vector.reduce_max(
                neg_mx, row[0:1, :], axis=mybir.AxisListType.X, negate=True
            )
            erow = row_pool.tile([1, s], f32, name=f"erow_{b}", tag="erow")
            tt_eng.tensor_tensor(erow, row[0:1, :], rowlo, mybir.AluOpType.add)
            prow = row_pool.tile([1, s], f32, name=f"prow_{b}", tag="prow")
            ssum = row_pool.tile([1, 1], f32, name=f"ssum_{b}", tag="ssum")
            nc.scalar.activation(
                prow,
                erow,
                mybir.ActivationFunctionType.Exp,
                bias=neg_mx,
                scale=1.0,
                accum_out=ssum,
            )
            rinv = row_pool.tile([1, 1], f32, name=f"rinv_{b}", tag="rinv")
            nc.vector.reciprocal(rinv, ssum)
            arow = row_pool.tile([1, s], f32, name=f"arow_{b}", tag="arow")
            tt_eng.tensor_scalar_mul(arow, prow, rinv)
            nc.gpsimd.dma_start(out=out_d[b : b + 1, :], in_=arow)

    nc.compile()
    return nc


def _split_hi_lo(x: np.ndarray):
    """fp32 -> (hi, lo) bf16 pair with hi + lo ~= x to ~18 mantissa bits."""
    import ml_dtypes

    hi = x.astype(ml_dtypes.bfloat16)
    lo = (x - hi.astype(np.float32)).astype(ml_dtypes.bfloat16)
    return hi, lo


def shard_inputs(
    hidden: np.ndarray,
    encoder_outputs: np.ndarray,
    g: int = G,
    n_cores: int = N_CORES,
):
    """Full inputs -> per-core input maps matching build_nc()'s DRAM layout."""
    s, b, d = encoder_outputs.shape
    b_loc = b // n_cores
    dc = d // P
    kg_cnt = dc // g

    # [S, B, D] -> [B, D, S] once (single big transpose), then per-core slices
    enc_bds = np.ascontiguousarray(
        np.asarray(encoder_outputs, dtype=np.float32).transpose(1, 2, 0)
    )
    ehi, elo = _split_hi_lo(enc_bds)  # [B, D, S] bf16 each
    hhi, hlo = _split_hi_lo(np.asarray(hidden[0], dtype=np.float32))  # [B, D]

    in_maps = []
    for c in range(n_cores):
        bs = slice(c * b_loc, (c + 1) * b_loc)
        # enc: [b_loc, kg, g, 128, 2, s]
        hi = ehi[bs].reshape(b_loc, kg_cnt, g, P, 1, s)
        lo = elo[bs].reshape(b_loc, kg_cnt, g, P, 1, s)
        enc_t = np.ascontiguousarray(np.concatenate([hi, lo], axis=4))
        # h: [128, b_loc*dc, 2]
        hh = hhi[bs].reshape(b_loc * dc, P, 1)
        hlevel = hlo[bs].reshape(b_loc * dc, P, 1)
        h_t = np.ascontiguousarray(
            np.concatenate([hh, hlevel], axis=2).transpose(1, 0, 2)
        )
        in_maps.append({"enc": enc_t, "h": h_t})
    return in_maps


_NC_CACHE: dict = {}


def _get_nc():
    if "nc" not in _NC_CACHE:
        _NC_CACHE["nc"] = build_nc()
    return _NC_CACHE["nc"]


def kernel(hidden: np.ndarray, encoder_outputs: np.ndarray) -> np.ndarray:
    hidden = np.asarray(hidden, dtype=np.float32)
    encoder_outputs = np.asarray(encoder_outputs, dtype=np.float32)
    assert hidden.shape == (1, B, D), hidden.shape
    assert encoder_outputs.shape == (S, B, D), encoder_outputs.shape

    nc = _get_nc()
    in_maps = shard_inputs(hidden, encoder_outputs)
    res = run_bass_kernel_spmd(nc, in_maps, core_ids=list(range(N_CORES)))
    attn = np.concatenate([res.results[c]["out"] for c in range(N_CORES)], axis=0)
    return attn[:, None, :].astype(np.float32)


# revision 19
# speedup vs baseline: 1.4423x; 1.4423x over previous
"""Trainium2 Bass kernel for batched dot-product attention + softmax.

Reference computation (all fp32):
    hidden:          [1, B=64, D=1024]
    encoder_outputs: [S=2048, B=64, D=1024]
    energies[b, s] = dot(hidden[0, b], encoder_outputs[s, b])   # [B, S]
    attn = softmax(energies, axis=-1)                           # [B, S]
    return attn[:, None, :]                                     # [B, 1, S]

Sharding: data-parallel over the batch dim -- each of the 8 NeuronCores
handles B_LOC = 8 batches. No cross-core communication (softmax is per-row).

Numerics: fp32 matmuls on the PE run at 4 cycles/row, which makes TensorE
(not HBM) the bottleneck. Instead each fp32 operand is split on the host
into a bf16 (hi, lo) pair: x = hi + lo + O(2^-18 x). Per d-chunk the PE
runs two bf16 matmuls (moving operand enc_hi then enc_lo) against the
2-column stationary [h_hi | h_lo], so PSUM rows {0,1} accumulate all four
cross products: row0 = e_hi.h_hi + e_lo.h_hi, row1 = e_hi.h_lo + e_lo.h_lo.
energies = row0 + row1 (added during the softmax stage). bf16 streams at
1 cycle/row => PE time halves; HBM traffic is unchanged (2+2 bytes/elem).

Per-core device layout (host-prepared):
    enc: [B_LOC, KG, 128, G*2, S] bf16 -- d on partitions; per (b, kg) a
         contiguous 4 MiB block holding G=4 d-chunks x (hi,lo) x S.
    h:   [128, B_LOC*DC, 2] bf16 -- (hi, lo) stationary column pairs.
"""

from contextlib import ExitStack

import numpy as np

import concourse.bacc as bacc
import concourse.bass as bass
import concourse.mybir as mybir
import concourse.tile as tile
from concourse.bass_utils import run_bass_kernel_spmd

N_CORES = 8
S = 2048
B = 64
D = 1024
P = 128
B_LOC = B // N_CORES  # 8 batches per core
DC = D // P  # 8 contraction chunks of 128
G = 4  # d-chunks per DMA (4 MiB transfers)
NBLK = 512  # moving-operand free dim per matmul (one fp32 PSUM bank)


def build_nc(
    b_loc: int = B_LOC,
    dc: int = DC,
    s: int = S,
    n_cores: int = N_CORES,
    g: int = G,
    enc_bufs: int = 3,
):
    """Build and compile the per-core Bass program (SPMD: same NEFF on all cores)."""
    assert dc % g == 0
    kg_cnt = dc // g
    nblk = min(NBLK, s)
    n_sblk = s // nblk

    nc = bacc.Bacc(
        "TRN2",
        target_bir_lowering=False,
        debug=False,
        num_devices=n_cores,
    )
    f32 = mybir.dt.float32
    bf16 = mybir.dt.bfloat16
    enc_d = nc.dram_tensor(
        "enc", [b_loc, kg_cnt, g, P, 2, s], bf16, kind="ExternalInput"
    ).ap()
    h_d = nc.dram_tensor("h", [P, b_loc * dc, 2], bf16, kind="ExternalInput").ap()
    out_d = nc.dram_tensor("out", [b_loc, s], f32, kind="ExternalOutput").ap()

    with ExitStack() as ctx:
        tc = ctx.enter_context(tile.TileContext(nc))
        enc_pool = ctx.enter_context(tc.tile_pool(name="enc_pool", bufs=enc_bufs))
        singles = ctx.enter_context(tc.tile_pool(name="singles", bufs=1))
        psum_pool = ctx.enter_context(
            tc.tile_pool(name="psum_pool", bufs=2, space="PSUM")
        )
        row_pool = ctx.enter_context(tc.tile_pool(name="row_pool", bufs=2))
        soft_pool = ctx.enter_context(tc.tile_pool(name="soft_pool", bufs=1))

        h_sb = singles.tile([P, b_loc * dc, 2], bf16)
        nc.sync.dma_start(out=h_sb, in_=h_d)

        # Alternate the two HWDGE rings (SP / ACT) across 1 MiB enc pieces.
        dma_engines = [nc.sync, nc.scalar]
        dma_idx = 0

        for b in range(b_loc):
            # psum rows {0, 1}: the M=2 stationary [h_hi | h_lo] makes each
            # moving stream (e_hi, then e_lo) hit both stationaries, so the
            # row sum holds all four cross products.
            psums = [
                psum_pool.tile([2, nblk], f32, name=f"ps_{b}_{j}", tag=f"ps{j}")
                for j in range(n_sblk)
            ]
            for kg in range(kg_cnt):
                # [128, g, 2, s] bf16 tile, filled by g 1-MiB DMA pieces so the
                # first matmuls start after ~1 MiB (subtile deps), spread over
                # both HWDGE rings for overlap.
                et = enc_pool.tile([P, g, 2, s], bf16, name=f"enc_{b}_{kg}", tag="enc")
                for gi in range(g):
                    eng = dma_engines[dma_idx % 2]
                    dma_idx += 1
                    eng.dma_start(out=et[:, gi], in_=enc_d[b, kg, gi])
                for gi in range(g):
                    k = kg * g + gi
                    col = b * dc + k
                    for j in range(n_sblk):
                        for hl in range(2):
                            nc.tensor.matmul(
                                psums[j][:, :],
                                lhsT=h_sb[:, col, :],
                                rhs=et[:, gi, hl, j * nblk : (j + 1) * nblk],
                                start=(k == 0 and hl == 0),
                                stop=(k == dc - 1 and hl == 1),
                            )
            row = row_pool.tile([2, s], f32, name=f"row_{b}", tag="row")
            for j in range(n_sblk):
                js = slice(j * nblk, (j + 1) * nblk)
                nc.vector.tensor_copy(row[:, js], psums[j])
            # fold lo row (partition 1) onto partition 0 via SBUF->SBUF DMA,
            # then run this batch's softmax entirely on partition 0 -- each
            # batch's chain overlaps the next batches' DMA/matmul stream.
            # The row max is taken from the hi row alone (lo row shifts it by
            # at most ~2^-9 |e|, which the normalization absorbs), so it runs
            # concurrently with the lo-row DMA + add.
            rowlo = row_pool.tile([1, s], f32, name=f"rowlo_{b}", tag="rowlo")
            nc.gpsimd.dma_start(out=rowlo, in_=row[1:2, :])
            neg_mx = row_pool.tile([1, 1], f32, name=f"mx_{b}", tag="mx")
            nc.vector.reduce_max(
                neg_mx, row[0:1, :], axis=mybir.AxisListType.X, negate=True
            )
            erow = row_pool.tile([1, s], f32, name=f"erow_{b}", tag="erow")
            nc.vector.tensor_tensor(erow, row[0:1, :], rowlo, mybir.AluOpType.add)
            prow = row_pool.tile([1, s], f32, name=f"prow_{b}", tag="prow")
            ssum = row_pool.tile([1, 1], f32, name=f"ssum_{b}", tag="ssum")
            nc.scalar.activation(
                prow,
                erow,
                mybir.ActivationFunctionType.Exp,
                bias=neg_mx,
                scale=1.0,
                accum_out=ssum,
            )
            rinv = row_pool.tile([1, 1], f32, name=f"rinv_{b}", tag="rinv")
            nc.vector.reciprocal(rinv, ssum)
            arow = row_pool.tile([1, s], f32, name=f"arow_{b}", tag="arow")
            nc.vector.tensor_scalar_mul(arow, prow, rinv)
            nc.gpsimd.dma_start(out=out_d[b : b + 1, :], in_=arow)

    nc.compile()
    return nc


def _split_hi_lo(x: np.ndarray):
    """fp32 -> (hi, lo) bf16 pair with hi + lo ~= x to ~18 mantissa bits."""
    import ml_dtypes

    hi = x.astype(ml_dtypes.bfloat16)
    lo = (x - hi.astype(np.float32)).astype(ml_dtypes.bfloat16)
    return hi, lo


def shard_inputs(
    hidden: np.ndarray,
    encoder_outputs: np.ndarray,
    g: int = G,
    n_cores: int = N_CORES,
):
    """Full inputs -> per-core input maps matching build_nc()'s DRAM layout."""
    s, b, d = encoder_outputs.shape
    b_loc = b // n_cores
    dc = d // P
    kg_cnt = dc // g

    # [S, B, D] -> [B, D, S] once (single big transpose), then per-core slices
    enc_bds = np.ascontiguousarray(
        np.asarray(encoder_outputs, dtype=np.float32).transpose(1, 2, 0)
    )
    ehi, elo = _split_hi_lo(enc_bds)  # [B, D, S] bf16 each
    hhi, hlo = _split_hi_lo(np.asarray(hidden[0], dtype=np.float32))  # [B, D]

    in_maps = []
    for c in range(n_cores):
        bs = slice(c * b_loc, (c + 1) * b_loc)
        # enc: [b_loc, kg, g, 128, 2, s]
        hi = ehi[bs].reshape(b_loc, kg_cnt, g, P, 1, s)
        lo = elo[bs].reshape(b_loc, kg_cnt, g, P, 1, s)
        enc_t = np.ascontiguousarray(np.concatenate([hi, lo], axis=4))
        # h: [128, b_loc*dc, 2]
        hh = hhi[bs].reshape(b_loc * dc, P, 1)
        hlevel = hlo[bs].reshape(b_loc * dc, P, 1)
        h_t = np.ascontiguousarray(
            np.concatenate([hh, hlevel], axis=2).transpose(1, 0, 2)
        )
        in_maps.append({"enc": enc_t, "h": h_t})
    return in_maps


_NC_CACHE: dict = {}


def _get_nc():
    if "nc" not in _NC_CACHE:
        _NC_CACHE["nc"] = build_nc()
    return _NC_CACHE["nc"]


def kernel(hidden: np.ndarray, encoder_outputs: np.ndarray) -> np.ndarray:
    hidden = np.asarray(hidden, dtype=np.float32)
    encoder_outputs = np.asarray(encoder_outputs, dtype=np.float32)
    assert hidden.shape == (1, B, D), hidden.shape
    assert encoder_outputs.shape == (S, B, D), encoder_outputs.shape

    nc = _get_nc()
    in_maps = shard_inputs(hidden, encoder_outputs)
    res = run_bass_kernel_spmd(nc, in_maps, core_ids=list(range(N_CORES)))
    attn = np.concatenate([res.results[c]["out"] for c in range(N_CORES)], axis=0)
    return attn[:, None, :].astype(np.float32)


# revision 20
# speedup vs baseline: 1.5937x; 1.1049x over previous
"""Trainium2 Bass kernel for batched dot-product attention + softmax.

Reference computation (all fp32):
    hidden:          [1, B=64, D=1024]
    encoder_outputs: [S=2048, B=64, D=1024]
    energies[b, s] = dot(hidden[0, b], encoder_outputs[s, b])   # [B, S]
    attn = softmax(energies, axis=-1)                           # [B, S]
    return attn[:, None, :]                                     # [B, 1, S]

Sharding: data-parallel over the batch dim -- each of the 8 NeuronCores
handles B_LOC = 8 batches. No cross-core communication (softmax is per-row).

Numerics: fp32 matmuls on the PE run at 4 cycles/row, which makes TensorE
(not HBM) the bottleneck. Instead each fp32 operand is split on the host
into a bf16 (hi, lo) pair: x = hi + lo + O(2^-18 x). Per d-chunk the PE
runs two bf16 matmuls (moving operand enc_hi then enc_lo) against the
2-column stationary [h_hi | h_lo], so PSUM rows {0,1} accumulate all four
cross products: row0 = e_hi.h_hi + e_lo.h_hi, row1 = e_hi.h_lo + e_lo.h_lo.
energies = row0 + row1 (added during the softmax stage). bf16 streams at
1 cycle/row => PE time halves; HBM traffic is unchanged (2+2 bytes/elem).

Per-core device layout (host-prepared):
    enc: [B_LOC, KG, 128, G*2, S] bf16 -- d on partitions; per (b, kg) a
         contiguous 4 MiB block holding G=4 d-chunks x (hi,lo) x S.
    h:   [128, B_LOC*DC, 2] bf16 -- (hi, lo) stationary column pairs.
"""

from contextlib import ExitStack

import numpy as np

import concourse.bacc as bacc
import concourse.bass as bass
import concourse.mybir as mybir
import concourse.tile as tile
from concourse.bass_utils import run_bass_kernel_spmd

N_CORES = 8
S = 2048
B = 64
D = 1024
P = 128
B_LOC = B // N_CORES  # 8 batches per core
DC = D // P  # 8 contraction chunks of 128
G = 4  # d-chunks per DMA (4 MiB transfers)
NBLK = 512  # moving-operand free dim per matmul (one fp32 PSUM bank)


def build_nc(
    b_loc: int = B_LOC,
    dc: int = DC,
    s: int = S,
    n_cores: int = N_CORES,
    g: int = G,
    enc_bufs: int = 4,
):
    """Build and compile the per-core Bass program (SPMD: same NEFF on all cores)."""
    assert dc % g == 0
    kg_cnt = dc // g
    nblk = min(NBLK, s)
    n_sblk = s // nblk

    nc = bacc.Bacc(
        "TRN2",
        target_bir_lowering=False,
        debug=False,
        num_devices=n_cores,
    )
    f32 = mybir.dt.float32
    bf16 = mybir.dt.bfloat16
    enc_d = nc.dram_tensor(
        "enc", [b_loc, kg_cnt, g, P, 2, s], bf16, kind="ExternalInput"
    ).ap()
    h_d = nc.dram_tensor("h", [P, b_loc * dc, 2], bf16, kind="ExternalInput").ap()
    out_d = nc.dram_tensor("out", [b_loc, s], f32, kind="ExternalOutput").ap()

    with ExitStack() as ctx:
        tc = ctx.enter_context(tile.TileContext(nc))
        enc_pool = ctx.enter_context(tc.tile_pool(name="enc_pool", bufs=enc_bufs))
        singles = ctx.enter_context(tc.tile_pool(name="singles", bufs=1))
        psum_pool = ctx.enter_context(
            tc.tile_pool(name="psum_pool", bufs=2, space="PSUM")
        )
        row_pool = ctx.enter_context(tc.tile_pool(name="row_pool", bufs=2))
        soft_pool = ctx.enter_context(tc.tile_pool(name="soft_pool", bufs=1))

        h_sb = singles.tile([P, b_loc * dc, 2], bf16)
        nc.sync.dma_start(out=h_sb, in_=h_d)

        # Alternate the two HWDGE rings (SP / ACT) across 1 MiB enc pieces.
        dma_engines = [nc.sync, nc.scalar]
        dma_idx = 0

        for b in range(b_loc):
            # psum rows {0, 1}: the M=2 stationary [h_hi | h_lo] makes each
            # moving stream (e_hi, then e_lo) hit both stationaries, so the
            # row sum holds all four cross products.
            psums = [
                psum_pool.tile([2, nblk], f32, name=f"ps_{b}_{j}", tag=f"ps{j}")
                for j in range(n_sblk)
            ]
            for kg in range(kg_cnt):
                # [128, g, 2, s] bf16 tile, filled by g 1-MiB DMA pieces so the
                # first matmuls start after ~1 MiB (subtile deps), spread over
                # both HWDGE rings for overlap.
                et = enc_pool.tile([P, g, 2, s], bf16, name=f"enc_{b}_{kg}", tag="enc")
                for gi in range(g):
                    eng = dma_engines[dma_idx % 2]
                    dma_idx += 1
                    eng.dma_start(out=et[:, gi], in_=enc_d[b, kg, gi])
                for gi in range(g):
                    k = kg * g + gi
                    col = b * dc + k
                    for j in range(n_sblk):
                        for hl in range(2):
                            nc.tensor.matmul(
                                psums[j][:, :],
                                lhsT=h_sb[:, col, :],
                                rhs=et[:, gi, hl, j * nblk : (j + 1) * nblk],
                                start=(k == 0 and hl == 0),
                                stop=(k == dc - 1 and hl == 1),
                            )
            row = row_pool.tile([2, s], f32, name=f"row_{b}", tag="row")
            for j in range(n_sblk):
                js = slice(j * nblk, (j + 1) * nblk)
                nc.vector.tensor_copy(row[:, js], psums[j])
            # fold lo row (partition 1) onto partition 0 via SBUF->SBUF DMA,
            # then run this batch's softmax entirely on partition 0 -- each
            # batch's chain overlaps the next batches' DMA/matmul stream.
            # The row max is taken from the hi row alone (lo row shifts it by
            # at most ~2^-9 |e|, which the normalization absorbs), so it runs
            # concurrently with the lo-row DMA + add.
            rowlo = row_pool.tile([1, s], f32, name=f"rowlo_{b}", tag="rowlo")
            nc.gpsimd.dma_start(out=rowlo, in_=row[1:2, :])
            neg_mx = row_pool.tile([1, 1], f32, name=f"mx_{b}", tag="mx")
            nc.vector.reduce_max(
                neg_mx, row[0:1, :], axis=mybir.AxisListType.X, negate=True
            )
            erow = row_pool.tile([1, s], f32, name=f"erow_{b}", tag="erow")
            nc.vector.tensor_tensor(erow, row[0:1, :], rowlo, mybir.AluOpType.add)
            ssum = row_pool.tile([1, 1], f32, name=f"ssum_{b}", tag="ssum")
            nc.scalar.activation(
                erow,
                erow,
                mybir.ActivationFunctionType.Exp,
                bias=neg_mx,
                scale=1.0,
                accum_out=ssum,
            )
            rinv = row_pool.tile([1, 1], f32, name=f"rinv_{b}", tag="rinv")
            nc.vector.reciprocal(rinv, ssum)
            nc.vector.tensor_scalar_mul(erow, erow, rinv)
            nc.gpsimd.dma_start(out=out_d[b : b + 1, :], in_=erow)

    nc.compile()
    return nc


def _split_hi_lo(x: np.ndarray):
    """fp32 -> (hi, lo) bf16 pair with hi + lo ~= x to ~18 mantissa bits."""
    import ml_dtypes

    hi = x.astype(ml_dtypes.bfloat16)
    lo = (x - hi.astype(np.float32)).astype(ml_dtypes.bfloat16)
    return hi, lo


def shard_inputs(
    hidden: np.ndarray,
    encoder_outputs: np.ndarray,
    g: int = G,
    n_cores: int = N_CORES,
):
    """Full inputs -> per-core input maps matching build_nc()'s DRAM layout."""
    s, b, d = encoder_outputs.shape
    b_loc = b // n_cores
    dc = d // P
    kg_cnt = dc // g

    # [S, B, D] -> [B, D, S] once (single big transpose), then per-core slices
    enc_bds = np.ascontiguousarray(
        np.asarray(encoder_outputs, dtype=np.float32).transpose(1, 2, 0)
    )
    ehi, elo = _split_hi_lo(enc_bds)  # [B, D, S] bf16 each
    hhi, hlo = _split_hi_lo(np.asarray(hidden[0], dtype=np.float32))  # [B, D]

    in_maps = []
    for c in range(n_cores):
        bs = slice(c * b_loc, (c + 1) * b_loc)
        # enc: [b_loc, kg, g, 128, 2, s]
        hi = ehi[bs].reshape(b_loc, kg_cnt, g, P, 1, s)
        lo = elo[bs].reshape(b_loc, kg_cnt, g, P, 1, s)
        enc_t = np.ascontiguousarray(np.concatenate([hi, lo], axis=4))
        # h: [128, b_loc*dc, 2]
        hh = hhi[bs].reshape(b_loc * dc, P, 1)
        hlevel = hlo[bs].reshape(b_loc * dc, P, 1)
        h_t = np.ascontiguousarray(
            np.concatenate([hh, hlevel], axis=2).transpose(1, 0, 2)
        )
        in_maps.append({"enc": enc_t, "h": h_t})
    return in_maps


_NC_CACHE: dict = {}


def _get_nc():
    if "nc" not in _NC_CACHE:
        _NC_CACHE["nc"] = build_nc()
    return _NC_CACHE["nc"]


def kernel(hidden: np.ndarray, encoder_outputs: np.ndarray) -> np.ndarray:
    hidden = np.asarray(hidden, dtype=np.float32)
    encoder_outputs = np.asarray(encoder_outputs, dtype=np.float32)
    assert hidden.shape == (1, B, D), hidden.shape
    assert encoder_outputs.shape == (S, B, D), encoder_outputs.shape

    nc = _get_nc()
    in_maps = shard_inputs(hidden, encoder_outputs)
    res = run_bass_kernel_spmd(nc, in_maps, core_ids=list(range(N_CORES)))
    attn = np.concatenate([res.results[c]["out"] for c in range(N_CORES)], axis=0)
    return attn[:, None, :].astype(np.float32)
